# revision 3
# baseline (speedup 1.0000x reference)
"""Trainium2 Bass kernel for the 4-layer GCN + mesh-unpool network
(nn_Net_7060926234635), distributed across 8 NeuronCores.

v2: dst-sharded graph parallelism with fully composed gather indices.
Every layer's aggregation gathers directly from a replicated table in
"global new id" order; unpool gathers are composed into the edge slot
indices on the host, and the layer weight matrices W3/W4 are folded into
the tables BEFORE replication (valid since row-gather commutes with the
feature matmul).  Per-slot dis[src] weights ride in fp32 weight grids.

Pipeline per core:
  L1 agg (table x0*?)        -> A1sh [12.5k,32]  -> AllGather A1f
  L2 agg (table A1f, u1-composed, wg=dis2[s])    -> A2sh [25k,64] local
  t3 build: A2sh @ W3        -> t3sh [25k,32]    -> AllGather t3f
  L3 agg (table t3f, u2-composed, wg=dis3[s])    -> A3sh [50k,32] local
  t4 build: A3sh @ W4p       -> t4sh [50k,4]     -> AllGather t4f
  L4 agg (table t4f, u3-composed, wg=dis4[s])    -> A4sh [100k,4]
  AllGather A4f -> final gather by u4            -> outbuf [200k,4]
"""
import sys
sys.path.insert(0, "/opt/trn_rl_repo")

import numpy as np

NC = 8
P = 128


def pad_to(x, m):
    return (x + m - 1) // m * m


# ----------------------------------------------------------------------------
# host-side planning
# ----------------------------------------------------------------------------

def make_dis(edge_index, n):
    deg = np.bincount(edge_index[1], minlength=n).astype(np.float64) + 1.0
    return (1.0 / np.sqrt(deg)).astype(np.float32)


def plan_agg(dst_old, slot_row, n, self_row, G, slot_weight=None,
             self_weight=None, zero_row=0, col_budget=128):
    """Padded-CSR slot grids over in-degree-sorted node shards.

    Returns groups/idx/w/dis-layouts exactly as v1 (see kernel.py)."""
    shard = n // NC
    shardP = pad_to(shard, P)
    ntiles = shardP // P
    c_of = dst_old // shard
    has_w = slot_weight is not None

    percore = []
    for c in range(NC):
        m = c_of == c
        dl = dst_old[m] - c * shard
        deg = np.bincount(dl, minlength=shard)
        perm = np.argsort(-deg, kind="stable")
        inv = np.empty_like(perm); inv[perm] = np.arange(shard)
        percore.append((m, dl, deg, perm, inv))

    def span_kt(lo_t, Gg):
        lo, hi = lo_t * P, min((lo_t + Gg) * P, shard)
        kt = 0
        for c in range(NC):
            deg_new = percore[c][2][percore[c][3]]
            if hi > lo:
                kt = max(kt, int(deg_new[lo:hi].max()))
        return kt + 1
    groups = []
    t = 0
    while t < ntiles:
        Gg = min(G, ntiles - t)
        kt = span_kt(t, Gg)
        while Gg > 1 and Gg * kt > col_budget:
            Gg = max(1, min(Gg - 1, col_budget // kt))
            kt = span_kt(t, Gg)
        groups.append((Gg, kt))
        t += Gg

    colbase = np.zeros(ntiles + 1, np.int64)
    kts_tile = []
    for (Gg, kt) in groups:
        kts_tile += [kt] * Gg
    for t in range(ntiles):
        colbase[t + 1] = colbase[t] + kts_tile[t]
    Ctot = int(colbase[-1])

    out = {"groups": groups, "idx": [], "w": [], "perm": [],
           "shardP": shardP, "Ctot": Ctot}
    sigma = np.empty(n, np.int64)
    for c in range(NC):
        m, dl, deg, perm, inv = percore[c]
        sigma[c * shard:(c + 1) * shard] = c * shardP + inv
        sr = slot_row[m]
        sw = slot_weight[m] if has_w else None
        nd = inv[dl]
        order = np.argsort(nd, kind="stable")
        nd_s = nd[order]; sr_s = sr[order]
        deg_new = deg[perm]
        starts = np.zeros(shard + 1, np.int64)
        np.cumsum(deg_new, out=starts[1:])
        idx = np.full((P, Ctot), zero_row, np.int64)
        w = np.zeros((P, Ctot), np.float32) if has_w else None
        r = np.arange(len(nd_s)) - starts[nd_s]
        pp_ = nd_s % P
        tt_ = nd_s // P
        cols = colbase[tt_] + r
        idx[pp_, cols] = sr_s
        if has_w:
            w[pp_, cols] = sw[order]
        v = np.arange(shard)
        scols = colbase[v // P] + deg_new[v]
        idx[v % P, scols] = self_row[perm + c * shard]
        if has_w:
            w[v % P, scols] = self_weight[perm + c * shard]
        out["idx"].append(idx.astype(np.int32))
        out["w"].append(w)
        out["perm"].append(perm)
    out["sigma"] = sigma
    return out


def tile_cols(vec_percore, shardP):
    outs = []
    for v in vec_percore:
        a = np.zeros(shardP, np.float32)
        a[:len(v)] = v
        outs.append(a.reshape(shardP // P, P).T.copy())
    return outs


def prep(inputs, G1=16, G2=4, G3=32, G4=32):
    x = np.asarray(inputs["x"], np.float32)
    W = [np.asarray(inputs[f"W{i}"], np.float32) for i in (1, 2, 3, 4)]
    ei = [np.asarray(inputs[f"edge_index{i}"]).astype(np.int64) for i in range(4)]
    u = [np.asarray(inputs[f"unpool{i}"]).astype(np.int64) for i in (1, 2, 3, 4)]
    n = [x.shape[0], len(u[0]), len(u[1]), len(u[2])]
    nout = len(u[3])
    dis = [make_dis(ei[l], n[l]) for l in range(4)]

    # L1: table = x0p (original order), weighted by dis1[src]
    L1 = plan_agg(ei[0][1], ei[0][0], n[0], self_row=np.arange(n[0]), G=G1,
                  slot_weight=dis[0][ei[0][0]], self_weight=dis[0])
    s1 = L1["sigma"]
    # L2: table = A1f (global new order), composed via u1, wg = dis2[s]
    L2 = plan_agg(ei[1][1], s1[u[0][ei[1][0]]], n[1], self_row=s1[u[0]], G=G2,
                  slot_weight=dis[1][ei[1][0]], self_weight=dis[1])
    s2 = L2["sigma"]
    # L3: table = t3f = (relu'd A2) @ W3 in s2 order, composed via u2
    L3 = plan_agg(ei[2][1], s2[u[1][ei[2][0]]], n[2], self_row=s2[u[1]], G=G3,
                  slot_weight=dis[2][ei[2][0]], self_weight=dis[2],
                  col_budget=240)
    s3 = L3["sigma"]
    # L4: table = t4f = A3 @ W4p in s3 order, composed via u3
    L4 = plan_agg(ei[3][1], s3[u[2][ei[3][0]]], n[3], self_row=s3[u[2]], G=G4,
                  slot_weight=dis[3][ei[3][0]], self_weight=dis[3],
                  col_budget=256)
    s4 = L4["sigma"]

    # final gather: core c's output rows are u4[c*shf:(c+1)*shf]
    shf = nout // NC
    CF = pad_to(shf, P) // P
    fidx = []
    for c in range(NC):
        rows = s4[u[3][c * shf:(c + 1) * shf]]
        g = np.zeros((P, CF), np.int64)
        k = np.arange(shf)
        g[k % P, k // P] = rows
        fidx.append(g.astype(np.int32))

    dis_dst = []
    for l, L in enumerate((L1, L2, L3, L4)):
        sh = n[l] // NC
        dis_dst.append(tile_cols(
            [dis[l][c * sh + L["perm"][c]] for c in range(NC)], L["shardP"]))

    meta = dict(
        n=n, nout=nout, shf=shf, CF=CF, G1=G1, G2=G2,
        L1=dict(groups=L1["groups"], shardP=L1["shardP"], C=L1["Ctot"],
                fin=4, fout=32, matmul=True),
        L2=dict(groups=L2["groups"], shardP=L2["shardP"], C=L2["Ctot"],
                fin=32, fout=64, matmul=True),
        L3=dict(groups=L3["groups"], shardP=L3["shardP"], C=L3["Ctot"],
                fin=32, fout=32, matmul=False),
        L4=dict(groups=L4["groups"], shardP=L4["shardP"], C=L4["Ctot"],
                fin=4, fout=4, matmul=False),
        b3=dict(fin=64, fout=32, G=2),     # t3 = A2 @ W3
        b4=dict(fin=32, fout=4, G=4),      # t4 = A3 @ W4p
        A1_rows=NC * L1["shardP"],
        t3_rows=NC * L2["shardP"],
        t4_rows=NC * L3["shardP"],
        A4_rows=NC * L4["shardP"],
    )

    x0p = np.zeros((n[0], 4), np.float32)
    x0p[:, :3] = x
    W1p = np.zeros((4, 32), np.float32); W1p[:3] = W[0]
    W4p = np.zeros((32, 4), np.float32); W4p[:, :3] = W[3]

    def blkdiag(Wm, G):
        fi, fo = Wm.shape
        B = np.zeros((G * fi, G * fo), np.float32)
        for g in range(G):
            B[g * fi:(g + 1) * fi, g * fo:(g + 1) * fo] = Wm
        return B

    ident = np.eye(P, dtype=np.float32)
    in_maps = []
    for c in range(NC):
        m = {
            "x0p": x0p, "ident": ident,
            "w1blk": blkdiag(W1p, G1), "w2blk": blkdiag(W[1], G2),
            "w3blk": blkdiag(W[2], meta["b3"]["G"]),
            "w4blk": blkdiag(W4p, meta["b4"]["G"]),
            "idx1": L1["idx"][c], "wg1": L1["w"][c], "dis1d": dis_dst[0][c],
            "idx2": L2["idx"][c], "wg2": L2["w"][c], "dis2d": dis_dst[1][c],
            "idx3": L3["idx"][c], "wg3": L3["w"][c], "dis3d": dis_dst[2][c],
            "idx4": L4["idx"][c], "wg4": L4["w"][c], "dis4d": dis_dst[3][c],
            "fidx": fidx[c],
        }
        in_maps.append(m)

    def reassemble(outs):
        res = np.empty((nout, 3), np.float32)
        for c in range(NC):
            res[c * shf:(c + 1) * shf] = outs[c]["outbuf"][:shf, :3]
        return res

    return meta, in_maps, reassemble


# ----------------------------------------------------------------------------
# device kernel builder
# ----------------------------------------------------------------------------

def build_kernel(meta):
    import concourse.bass as bass
    import concourse.mybir as mybir
    from concourse.bacc import Bacc
    from concourse.tile import TileContext
    from concourse.bass import IndirectOffsetOnAxis

    f32 = mybir.dt.float32
    i32 = mybir.dt.int32
    n = meta["n"]

    nc = Bacc("TRN2", target_bir_lowering=False, debug=False, num_devices=NC)

    x0p = nc.dram_tensor("x0p", [n[0], 4], f32, kind="ExternalInput")
    ident_d = nc.dram_tensor("ident", [P, P], f32, kind="ExternalInput")
    G1, G2 = meta["G1"], meta["G2"]
    w1blk_d = nc.dram_tensor("w1blk", [G1 * 4, G1 * 32], f32, kind="ExternalInput")
    w2blk_d = nc.dram_tensor("w2blk", [G2 * 32, G2 * 64], f32, kind="ExternalInput")
    w3blk_d = nc.dram_tensor("w3blk", [meta["b3"]["G"] * 64, meta["b3"]["G"] * 32],
                             f32, kind="ExternalInput")
    w4blk_d = nc.dram_tensor("w4blk", [meta["b4"]["G"] * 32, meta["b4"]["G"] * 4],
                             f32, kind="ExternalInput")

    def grid_in(name, C, dt=i32):
        return nc.dram_tensor(name, [P, C], dt, kind="ExternalInput")

    L1, L2, L3, L4 = meta["L1"], meta["L2"], meta["L3"], meta["L4"]
    idx1 = grid_in("idx1", L1["C"]); wg1 = grid_in("wg1", L1["C"], f32)
    dis1d = grid_in("dis1d", L1["shardP"] // P, f32)
    idx2 = grid_in("idx2", L2["C"]); wg2 = grid_in("wg2", L2["C"], f32)
    dis2d = grid_in("dis2d", L2["shardP"] // P, f32)
    idx3 = grid_in("idx3", L3["C"]); wg3 = grid_in("wg3", L3["C"], f32)
    dis3d = grid_in("dis3d", L3["shardP"] // P, f32)
    idx4 = grid_in("idx4", L4["C"]); wg4 = grid_in("wg4", L4["C"], f32)
    dis4d = grid_in("dis4d", L4["shardP"] // P, f32)
    fidx = grid_in("fidx", meta["CF"])

    outbuf = nc.dram_tensor("outbuf", [meta["CF"] * P, 4], f32,
                            kind="ExternalOutput")

    rg = [list(range(NC))]

    with TileContext(nc) as tc:
        with (
            tc.tile_pool(name="dramp", bufs=1, space="DRAM") as dramp,
            tc.tile_pool(name="consts", bufs=1) as constp,
            tc.tile_pool(name="idxp", bufs=3) as idxp,
            tc.tile_pool(name="gath", bufs=3) as gathp,
            tc.tile_pool(name="work", bufs=3) as workp,
            tc.tile_pool(name="outp", bufs=2) as outp,
            tc.tile_pool(name="psumT", bufs=2, space="PSUM") as psumTp,
            tc.tile_pool(name="psumM", bufs=2, space="PSUM") as psumMp,
        ):
            A1sh = dramp.tile([L1["shardP"], 32], f32)
            A1f = dramp.tile([meta["A1_rows"], 32], f32, addr_space="Shared")
            A2sh = dramp.tile([L2["shardP"], 64], f32)
            t3sh = dramp.tile([L2["shardP"], 32], f32)
            t3f = dramp.tile([meta["t3_rows"], 32], f32, addr_space="Shared")
            A3sh = dramp.tile([L3["shardP"], 32], f32)
            t4sh = dramp.tile([L3["shardP"], 4], f32)
            t4f = dramp.tile([meta["t4_rows"], 4], f32, addr_space="Shared")
            A4sh = dramp.tile([L4["shardP"], 4], f32)
            A4f = dramp.tile([meta["A4_rows"], 4], f32, addr_space="Shared")

            ident = constp.tile([P, P], f32)
            nc.sync.dma_start(out=ident[:], in_=ident_d[:, :])
            w1b = constp.tile([G1 * 4, G1 * 32], f32)
            nc.sync.dma_start(out=w1b[:], in_=w1blk_d[:, :])
            w2b = constp.tile([G2 * 32, G2 * 64], f32)
            nc.sync.dma_start(out=w2b[:], in_=w2blk_d[:, :])
            w3b = constp.tile([meta["b3"]["G"] * 64, meta["b3"]["G"] * 32], f32)
            nc.sync.dma_start(out=w3b[:], in_=w3blk_d[:, :])
            w4b = constp.tile([meta["b4"]["G"] * 32, meta["b4"]["G"] * 4], f32)
            nc.sync.dma_start(out=w4b[:], in_=w4blk_d[:, :])

            def agg_phase(lm, idx_d, w_d, disd_d, table_ap, out_dram, wblk, tag):
                fin, fout = lm["fin"], lm["fout"]
                t0 = 0
                col = 0
                for gi, (Gg, kt) in enumerate(lm["groups"]):
                    ncols = Gg * kt
                    idxt = idxp.tile([P, ncols], i32, tag=f"idx{tag}",
                                     name=f"idx{tag}_{gi}")
                    nc.sync.dma_start(out=idxt[:], in_=idx_d[:, col:col + ncols])
                    gt = gathp.tile([P, ncols * fin], f32, tag=f"g{tag}",
                                    name=f"g{tag}_{gi}")
                    for j in range(ncols):
                        nc.gpsimd.indirect_dma_start(
                            out=gt[:, j * fin:(j + 1) * fin],
                            out_offset=None,
                            in_=table_ap,
                            in_offset=IndirectOffsetOnAxis(
                                ap=idxt[:, j:j + 1], axis=0),
                        )
                    dcol = idxp.tile([P, Gg], f32, tag=f"d{tag}",
                                     name=f"d{tag}_{gi}")
                    nc.sync.dma_start(out=dcol[:], in_=disd_d[:, t0:t0 + Gg])
                    wt = idxp.tile([P, ncols], f32, tag=f"w{tag}",
                                   name=f"w{tag}_{gi}")
                    nc.sync.dma_start(out=wt[:], in_=w_d[:, col:col + ncols])
                    nc.vector.tensor_tensor(
                        out=gt[:].rearrange("p (c f) -> p c f", f=fin),
                        in0=gt[:].rearrange("p (c f) -> p c f", f=fin),
                        in1=wt[:].rearrange("p c -> p c").to_broadcast(
                            [P, ncols, fin]),
                        op=mybir.AluOpType.mult)
                    S = workp.tile([P, Gg * fin], f32, tag=f"S{tag}",
                                   name=f"S{tag}_{gi}")
                    nc.vector.tensor_reduce(
                        out=S[:].rearrange("p (g f) -> p g f", f=fin),
                        in_=gt[:].rearrange("p (g k f) -> p g f k",
                                            g=Gg, k=kt),
                        axis=mybir.AxisListType.X, op=mybir.AluOpType.add)
                    nc.vector.tensor_tensor(
                        out=S[:].rearrange("p (g f) -> p g f", f=fin),
                        in0=S[:].rearrange("p (g f) -> p g f", f=fin),
                        in1=dcol[:].to_broadcast([P, Gg, fin]),
                        op=mybir.AluOpType.mult)
                    if lm["matmul"]:
                        pT = psumTp.tile([Gg * fin, P], f32, tag="pT",
                                         name=f"pT{tag}_{gi}")
                        nc.tensor.transpose(out=pT[:], in_=S[:],
                                            identity=ident[:])
                        ST = workp.tile([Gg * fin, P], f32, tag=f"ST{tag}",
                                        name=f"ST{tag}_{gi}")
                        nc.scalar.copy(out=ST[:], in_=pT[:])
                        pM = psumMp.tile([P, Gg * fout], f32, tag="pM",
                                         name=f"pM{tag}_{gi}")
                        nc.tensor.matmul(out=pM[:], lhsT=ST[:],
                                         rhs=wblk[:Gg * fin, :Gg * fout],
                                         start=True, stop=True)
                        at = outp.tile([P, Gg * fout], f32, tag=f"A{tag}",
                                       name=f"A{tag}_{gi}")
                        nc.scalar.activation(
                            out=at[:], in_=pM[:],
                            func=mybir.ActivationFunctionType.Relu)
                    else:
                        at = outp.tile([P, Gg * fout], f32, tag=f"A{tag}",
                                       name=f"A{tag}_{gi}")
                        nc.scalar.activation(
                            out=at[:], in_=S[:],
                            func=mybir.ActivationFunctionType.Relu)
                    nc.sync.dma_start(
                        out=out_dram[t0 * P:(t0 + Gg) * P, :].rearrange(
                            "(g p) f -> p g f", p=P),
                        in_=at[:])
                    t0 += Gg
                    col += ncols

            def build_phase(src_dram, out_dram, fin, fout, G, wblk, nrows, tag):
                """out = src @ W (no relu; src already relu'd)."""
                ntile = nrows // P
                t0 = 0
                while t0 < ntile:
                    Gg = min(G, ntile - t0)
                    xt = workp.tile([P, Gg * fin], f32, tag=f"x{tag}",
                                    name=f"x{tag}_{t0}")
                    nc.sync.dma_start(
                        out=xt[:],
                        in_=src_dram[t0 * P:(t0 + Gg) * P, :].rearrange(
                            "(g p) f -> p g f", p=P))
                    pT = psumTp.tile([Gg * fin, P], f32, tag="pT",
                                     name=f"pTb{tag}_{t0}")
                    nc.tensor.transpose(out=pT[:], in_=xt[:], identity=ident[:])
                    ST = workp.tile([Gg * fin, P], f32, tag=f"STb{tag}",
                                    name=f"STb{tag}_{t0}")
                    nc.scalar.copy(out=ST[:], in_=pT[:])
                    pM = psumMp.tile([P, Gg * fout], f32, tag="pM",
                                     name=f"pMb{tag}_{t0}")
                    nc.tensor.matmul(out=pM[:], lhsT=ST[:],
                                     rhs=wblk[:Gg * fin, :Gg * fout],
                                     start=True, stop=True)
                    gt = outp.tile([P, Gg * fout], f32, tag=f"gb{tag}",
                                   name=f"gb{tag}_{t0}")
                    nc.scalar.copy(out=gt[:], in_=pM[:])
                    nc.sync.dma_start(
                        out=out_dram[t0 * P:(t0 + Gg) * P, :].rearrange(
                            "(g p) f -> p g f", p=P),
                        in_=gt[:])
                    t0 += Gg

            def fetch_final(idx_d, table_ap, out_dram):
                CF = meta["CF"]
                GS = 8
                col = 0
                while col < CF:
                    g = min(GS, CF - col)
                    idxt = idxp.tile([P, g], i32, tag="fidx",
                                     name=f"fidx_{col}")
                    nc.sync.dma_start(out=idxt[:], in_=idx_d[:, col:col + g])
                    gt = gathp.tile([P, g * 4], f32, tag="fg",
                                    name=f"fg_{col}")
                    for j in range(g):
                        nc.gpsimd.indirect_dma_start(
                            out=gt[:, j * 4:(j + 1) * 4],
                            out_offset=None,
                            in_=table_ap,
                            in_offset=IndirectOffsetOnAxis(
                                ap=idxt[:, j:j + 1], axis=0),
                        )
                    nc.sync.dma_start(
                        out=out_dram[col * P:(col + g) * P, :].rearrange(
                            "(g p) f -> p g f", p=P),
                        in_=gt[:])
                    col += g

            # ================= the program =================
            agg_phase(L1, idx1, wg1, dis1d, x0p[:, :], A1sh[:], w1b, "1")
            nc.gpsimd.collective_compute(
                "AllGather", mybir.AluOpType.bypass, replica_groups=rg,
                ins=[A1sh[:]], outs=[A1f[:]])
            agg_phase(L2, idx2, wg2, dis2d, A1f[:], A2sh[:], w2b, "2")
            build_phase(A2sh[:], t3sh[:], 64, 32, meta["b3"]["G"], w3b,
                        L2["shardP"], "3")
            nc.gpsimd.collective_compute(
                "AllGather", mybir.AluOpType.bypass, replica_groups=rg,
                ins=[t3sh[:]], outs=[t3f[:]])
            agg_phase(L3, idx3, wg3, dis3d, t3f[:], A3sh[:], None, "L3")
            build_phase(A3sh[:], t4sh[:], 32, 4, meta["b4"]["G"], w4b,
                        L3["shardP"], "4")
            nc.gpsimd.collective_compute(
                "AllGather", mybir.AluOpType.bypass, replica_groups=rg,
                ins=[t4sh[:]], outs=[t4f[:]])
            agg_phase(L4, idx4, wg4, dis4d, t4f[:], A4sh[:], None, "L4")
            nc.gpsimd.collective_compute(
                "AllGather", mybir.AluOpType.bypass, replica_groups=rg,
                ins=[A4sh[:]], outs=[A4f[:]])
            fetch_final(fidx, A4f[:], outbuf[:, :])

    nc.finalize()
    return nc


# ----------------------------------------------------------------------------
# PJRT runner (same as v1)
# ----------------------------------------------------------------------------
import numpy as np
import jax
from jax.sharding import Mesh, PartitionSpec, NamedSharding
from jax.experimental.shard_map import shard_map
from concourse import mybir
from concourse.bass2jax import _bass_exec_p, partition_id_tensor, install_neuronx_cc_hook


def make_runner(nc, n_cores=8):
    install_neuronx_cc_hook()
    partition_name = nc.partition_id_tensor.name if nc.partition_id_tensor else None
    in_names, out_names, out_avals = [], [], []
    for alloc in nc.m.functions[0].allocations:
        if not isinstance(alloc, mybir.MemoryLocationSet):
            continue
        name = alloc.memorylocations[0].name
        if alloc.kind == "ExternalInput":
            if name != partition_name:
                in_names.append(name)
        elif alloc.kind == "ExternalOutput":
            out_names.append(name)
            out_avals.append(jax.core.ShapedArray(
                tuple(alloc.tensor_shape), mybir.dt.np(alloc.dtype)))
    n_params = len(in_names)
    all_in_names = list(in_names) + list(out_names)
    if partition_name is not None:
        all_in_names.append(partition_name)

    def _body(*args):
        operands = list(args)
        if partition_name is not None:
            operands.append(partition_id_tensor())
        outs = _bass_exec_p.bind(
            *operands,
            out_avals=tuple(out_avals), in_names=tuple(all_in_names),
            out_names=tuple(out_names), lowering_input_output_aliases=(),
            sim_require_finite=False, sim_require_nnan=False, nc=nc)
        return tuple(outs)

    devices = jax.devices()[:n_cores]
    mesh = Mesh(np.asarray(devices), ("core",))
    n_outs = len(out_avals)
    in_specs = (PartitionSpec("core"),) * (n_params + n_outs)
    out_specs = (PartitionSpec("core"),) * len(out_names)
    sharded = jax.jit(shard_map(_body, mesh=mesh, in_specs=in_specs,
                                out_specs=out_specs, check_rep=False),
                      keep_unused=True)
    sharding = NamedSharding(mesh, PartitionSpec("core"))

    state = {}

    def prepare(in_maps):
        per_core = [[np.asarray(m[name]) for name in in_names] for m in in_maps]
        concat_in = [np.concatenate([per_core[c][i] for c in range(n_cores)], axis=0)
                     for i in range(n_params)]
        zeros = [np.zeros((n_cores * av.shape[0], *av.shape[1:]), av.dtype)
                 for av in out_avals]
        state["dev_in"] = [jax.device_put(a, sharding) for a in concat_in + zeros]
        jax.block_until_ready(state["dev_in"])

    def run():
        out = jax.block_until_ready(sharded(*state["dev_in"]))
        return out

    def fetch(out_arrs):
        return [
            {name: np.asarray(out_arrs[i]).reshape(n_cores, *out_avals[i].shape)[c]
             for i, name in enumerate(out_names)}
            for c in range(n_cores)
        ]

    return prepare, run, fetch


# ----------------------------------------------------------------------------
# public entry point
# ----------------------------------------------------------------------------
_CACHE = {}


def kernel(**inputs):
    import numpy as np
    for b in ("b1", "b2", "b3", "b4"):
        if b in inputs:
            assert not np.asarray(inputs[b]).any(), (
                "this kernel build assumes zero biases (per problem spec)")
    meta, in_maps, reassemble = prep(inputs)
    key = "k"
    if key not in _CACHE:
        nc = build_kernel(meta)
        _CACHE[key] = make_runner(nc)
    prepare, run, fetch = _CACHE[key]
    prepare(in_maps)
    outs = fetch(run())
    return reassemble(outs).astype(np.float32)
